# revision 26
# baseline (speedup 1.0000x reference)
"""Paged-KV-cache GQA attention with int8 tensor-cast quantization, TRN2.

Sharding: tensor-parallel over KV heads. Core c owns kv-head c and its G=4
query heads. Host side does only index-driven data movement (scatter new
quantized K/V into the paged cache, gather pages via block_table) plus the
int8 quantization of Q/K/V, all exact integer ops. The device kernel does
all the attention compute per (b, row) where row = (g, lqc) is a 128-query
block:

  scores[q, k] = sum_d Q[q,d] * K[k,d]           (PE, fp16 exact-int matmul
                                                  into 3 rotating PSUM regions)
  e = exp(C * scores)                            (ACT, PSUM->SBUF fp32,
                                                  accum_out -> row sums; the
                                                  causal tail gets raw exp,
                                                  masked + summed on DVE)
  p127 = RNE(e * 127/sum) -> int16 slab          (ONE DVE pass: the fp32->
                                                  int16 saturating convert IS
                                                  round-nearest-even)
  slab transpose p[q, k] -> pT[k, (c,row,q)]     (xbar DMA, int16 bytes, two
                                                  pieces per 4-row slab)
  in-place int16->fp16 convert of pT halves      (DVE, at the consuming row)
  outT[d, (row,q)] += V_c^T @ pT_c               (PE, V-stationary N=512,
                                                  interleaved with the next
                                                  batch's QK rows)

All matmul operands are integers held exactly in fp16 (|x| <= 128), so the
fp32 PSUM dot products are exact; the int16 convert reproduces
round-to-nearest-even of jnp.round for the prob quantization,
bit-identically to the reference chain (measured rel err 0.0).
"""

import sys

sys.path.insert(0, "/opt/trn_rl_repo")

import numpy as np

import concourse.bass as bass
import concourse.mybir as mybir
from concourse import tile
from concourse.bass_utils import run_bass_kernel_spmd

# Problem dims (hardcoded per spec)
B, H, KVH, D = 4, 32, 8, 128
LQ, S, BLOCK = 256, 4096, 16
BPS = S // BLOCK
NUM_BLOCKS = B * BPS
SLOTS = NUM_BLOCKS * BLOCK
T = B * LQ
G = H // KVH
N_CORES = 8
NCH = S // 128  # 32 partition-chunks of the key axis
NROW = G * (LQ // 128)  # 8 query rows of 128 per (b, core)
SM_SCALE = 1.0 / float(np.sqrt(D))
MAGIC = float(np.float32(12582912.0))  # 1.5 * 2**23: fp32 RNE round trick

F16 = np.float16
GS_SLAB = False  # gpsimd tensor_scalar measured 25x slower than DVE

_CACHE = {}


def _build(C, inv_ps, out_scale):
    nc = bass.Bass()
    f16 = mybir.dt.float16
    f32 = mybir.dt.float32
    i32 = mybir.dt.int32
    i16 = mybir.dt.int16
    AF = mybir.ActivationFunctionType
    OP = mybir.AluOpType
    X = mybir.AxisListType.X

    KQCOL = S + G * LQ

    kq = nc.dram_tensor("kq", [B, 128, KQCOL], f16, kind="ExternalInput")
    vv = nc.dram_tensor("vv", [B, 128, NCH * D], f16, kind="ExternalInput")
    # outT[b, d, row*128 + q] (row = g*2 + lqc), host transposes back
    out = nc.dram_tensor("out", [B, 128, NROW * 128], f32, kind="ExternalOutput")

    with tile.TileContext(nc) as tc:
        with (
            tc.tile_pool(name="const", bufs=1) as constp,
            tc.tile_pool(name="kqp", bufs=2) as kqp,
            tc.tile_pool(name="vp", bufs=2) as vp,
            tc.tile_pool(name="expf", bufs=2) as expp,
            tc.tile_pool(name="pslab", bufs=1) as pbfp,
            tc.tile_pool(name="ptslab", bufs=1) as ptp,
            tc.tile_pool(name="small", bufs=8) as smallp,
            tc.tile_pool(name="osb", bufs=1) as outp,
            tc.tile_pool(name="psum", bufs=1, space="PSUM") as psp,
        ):
            # additive causal masks for the last LQ key columns, one per
            # 128-row query half: keep iff q - j + lqc*128 >= 0
            ramp = constp.tile([128, 256], i32)
            nc.gpsimd.iota(ramp[:], pattern=[[-1, 256]], base=0, channel_multiplier=1)
            mask01 = []  # keep=1.0 / masked=0.0, for the last 256 key cols
            for lqc in range(LQ // 128):
                t = constp.tile([128, 256], f32, tag=f"mt{lqc}")
                nc.vector.tensor_scalar(
                    t[:], ramp[:], float(lqc * 128), 0.0, OP.add, OP.is_ge
                )
                mask01.append(t)

            # PSUM: sc = 3 rotating [128,1024] score regions (banks 0-5),
            # pv = 2 slab accumulators [128,512] (banks 6,7)
            sc = psp.tile([128, 3072], f32, tag="sc")
            pv = psp.tile([128, 1024], f32, tag="pv")

            kq_t = [None, None]
            v_t = [None, None]  # [cur b, prev b]
            pbf_t = [None, None]  # slab tiles of current b
            pt_t = [None, None]  # transposed slabs of previous b
            rctr = 0  # global score-region counter

            def emit_row(b, r, bt):
                """QK + softmax + quantize for query row r of batch b."""
                nonlocal rctr
                g, lqc = r // 2, r % 2
                qa = S + g * LQ + lqc * 128
                slots = [(rctr + j) % 3 for j in range(4)]
                rctr += 4

                def qk(j, half):
                    cs = j * 2 + half
                    col = slots[j] * 1024 + half * 512
                    nc.tensor.matmul(
                        sc[:, col : col + 512],
                        bt[:, qa : qa + 128],
                        bt[:, cs * 512 : (cs + 1) * 512],
                        start=True,
                        stop=True,
                        skip_group_check=True,
                    )

                # exp op plans: (src_col, width, e_col, accum?) — slot j3 ==
                # j0 always (3 regions, 4 uses), so exp#1 MUST be emitted
                # before the j3 matmuls reuse that region. Finer exp
                # granularity lets regions recycle piecewise (dissolves the
                # PE<->ACT region ping-pong). The last 256 key cols (causal
                # zone) get raw exp with NO accum; the mask + partial sum for
                # them run on DVE in SBUF, so no ACT op waits on a DVE op.
                if slots[0] == 2:  # pair (2,0) is address-non-contiguous
                    ops1 = [(2048, 1024, 0, 1), (0, 1024, 1024, 1)]
                else:
                    ops1 = [(slots[0] * 1024, 2048, 0, 1)]
                if slots[2] == 2:  # (slots[2], slots[3]) = (2, 0): split
                    ops2 = [(2048, 1024, 2048, 1), (0, 768, 3072, 1)]
                else:  # slots[3] == slots[2]+1: one contiguous 1792 read
                    ops2 = [(slots[2] * 1024, 1792, 2048, 1)]
                ops2 += [(slots[3] * 1024 + 768, 256, 3840, 0)]

                ef = expp.tile([128, S], f32, tag="ef")
                nsum = sum(o[3] for o in ops1 + ops2) + 1
                acc = smallp.tile([128, nsum], f32, tag="acc")
                nacc = 0

                def exp_ops(ops):
                    nonlocal nacc
                    for sc0, w, ec, has_acc in ops:
                        ao = None
                        if has_acc:
                            ao = acc[:, nacc : nacc + 1]
                            nacc += 1
                        nc.scalar.activation(
                            ef[:, ec : ec + w],
                            sc[:, sc0 : sc0 + w],
                            AF.Exp,
                            scale=C,
                            accum_out=ao,
                        )

                qk(0, 0); qk(0, 1); qk(1, 0); qk(1, 1)
                exp_ops(ops1)
                qk(2, 0); qk(2, 1); qk(3, 0); qk(3, 1)
                exp_ops(ops2)

                # causal mask: zero the masked tail of e, sum the kept part
                nc.vector.tensor_mul(
                    ef[:, 3840:4096], ef[:, 3840:4096], mask01[lqc][:]
                )
                nc.vector.tensor_reduce(
                    acc[:, nsum - 1 : nsum], ef[:, 3840:4096], X, OP.add
                )

                sumv = smallp.tile([128, 1], f32, tag="sumv")
                nc.vector.tensor_reduce(sumv[:], acc[:], X, OP.add)
                rv = smallp.tile([128, 1], f32, tag="rv")
                nc.vector.reciprocal(rv[:], sumv[:])
                r127 = smallp.tile([128, 1], f32, tag="r127")
                nc.vector.tensor_scalar_mul(r127[:], rv[:], inv_ps)

                # p127 = RNE(e * r127): the fp32->int16 saturating convert
                # rounds to nearest-even, straight to the slab; split into 2
                # chunk-halves so slab transposes can start piecewise
                s, gl = r // 4, r % 4
                for j in range(2):
                    nc.vector.tensor_scalar(
                        pbf_t[s][:, 16 * j : 16 * (j + 1), gl * 128 : (gl + 1) * 128],
                        ef[:, 2048 * j : 2048 * (j + 1)],
                        r127[:],
                        None,
                        OP.mult,
                    )

            def emit_pv(bprev, r, vtprev):
                """8 PV chunks of slab r//4 of the previous batch."""
                s = r // 4
                for c in range(8 * (r % 4), 8 * (r % 4) + 8):
                    nc.tensor.matmul(
                        pv[:, s * 512 : (s + 1) * 512],
                        vtprev[:, c * D : (c + 1) * D],
                        pt_t[s][:, c * 4 : (c + 1) * 4, :],
                        start=(c == 0),
                        stop=(c == NCH - 1),
                        skip_group_check=True,
                    )

            for b in range(B + 1):
                if b < B:
                    if b == 0:
                        bt = kqp.tile([128, KQCOL], f16, tag="kq", name="kqt")
                        nc.gpsimd.dma_start(out=bt[:], in_=kq[0])
                        kq_t = [bt, None]
                    bt = kq_t[0]
                    if b + 1 < B:
                        nxt = kqp.tile([128, KQCOL], f16, tag="kq", name="kqn")
                        nc.gpsimd.dma_start(out=nxt[:], in_=kq[b + 1])
                        kq_t = [nxt, None]
                    vt = vp.tile([128, NCH * D], f16, tag="v", name="vt")
                    nc.gpsimd.dma_start(out=vt[:], in_=vv[b])
                    v_t[1] = v_t[0]
                    v_t[0] = vt
                    pbf_t[0] = pbfp.tile([128, NCH, 512], i16, tag="pb0", name="pb0")
                    pbf_t[1] = pbfp.tile([128, NCH, 512], i16, tag="pb1", name="pb1")

                for r in range(NROW):
                    if b > 0:
                        # int16->fp16 convert of the pt half the next two
                        # rows' PV consume (fine-grained: never head-of-line
                        # blocks the DVE queue on the slab transpose).
                        # PV is emitted BEFORE this row's QK: PV chunks have
                        # no score-region deps, so the in-order PE queue
                        # fills region-wait gaps with PV work and stays
                        # HAM-warm for the QK burst.
                        s, j = r // 4, r % 4
                        if j % 2 == 0:
                            h = j // 2
                            pc = pt_t[s][:, h * 64 : (h + 1) * 64, :]
                            nc.vector.tensor_copy(pc, pc.bitcast(i16))
                        emit_pv(b - 1, r, v_t[1] if b <= B - 1 else v_t[0])
                    if b < B:
                        emit_row(b, r, bt)
                    # slab complete -> transpose it (int16 bytes) in 4
                    # chunk-quarter pieces for pipelining with cast/PV
                    if b < B and r % 4 == 3:
                        s = r // 4
                        pt_new = ptp.tile([128, 128, 128], f16, tag=f"pt{s}")
                        for j in range(2):
                            nc.sync.dma_start_transpose(
                                out=pt_new[:, 64 * j : 64 * (j + 1), :].bitcast(i16),
                                in_=pbf_t[s][:, 16 * j : 16 * (j + 1), :],
                            )
                        pt_t[s] = pt_new

                if b > 0:
                    ob = outp.tile([128, 1024], f32)
                    nc.vector.tensor_scalar_mul(ob[:], pv[:], out_scale)
                    nc.gpsimd.dma_start(out=out[b - 1], in_=ob[:])



    _legalize_waits(nc)
    return nc


def _legalize_waits(nc, maxw=1):
    """Walrus rejects instructions with too many sync waits. Move excess
    waits onto injected same-engine NoOps placed just before the
    instruction (engine program order preserved, so semantics identical)."""
    fixid = 0
    for bb in nc.main_func.blocks:
        insts = list(bb.instructions)
        changed = False
        newlist = []
        for ins in insts:
            si = ins.sync_info
            waits = list(si.on_wait) if si and si.on_wait else []
            if len(waits) > maxw:
                keep = waits[-maxw:]
                excess = waits[:-maxw]
                for j in range(0, len(excess), maxw):
                    nop = mybir.InstNoOp(name=f"I-waitfix-{fixid}", ins=[], outs=[])
                    fixid += 1
                    nop.engine = ins.engine
                    nop.sync_info = mybir.SyncInfo(
                        on_wait=excess[j : j + maxw], on_update=[]
                    )
                    newlist.append(nop)
                ins.sync_info = mybir.SyncInfo(
                    on_wait=keep,
                    on_update=list(si.on_update) if si.on_update else [],
                )
                changed = True
            newlist.append(ins)
        if changed:
            try:
                bb.instructions = newlist
            except Exception:
                bb.instructions.clear()
                bb.instructions.extend(newlist)
    return nc


def kernel(
    query,
    key,
    value,
    kv_cache,
    block_table,
    slot_mapping,
    query_start_loc,
    seq_lens,
    query_lens,
    q_scale,
    q_offset,
    kv_scale,
    kv_offset,
    prob_scale,
    prob_offset,
):
    query = np.asarray(query, np.float32)
    key = np.asarray(key, np.float32)
    value = np.asarray(value, np.float32)
    kv_cache = np.asarray(kv_cache)
    block_table = np.asarray(block_table, np.int32)
    slot_mapping = np.asarray(slot_mapping, np.int32)
    seq_lens = np.asarray(seq_lens, np.int32)
    qs = float(np.asarray(q_scale).reshape(-1)[0])
    qo = float(np.asarray(q_offset).reshape(-1)[0])
    ks = float(np.asarray(kv_scale).reshape(-1)[0])
    ko = float(np.asarray(kv_offset).reshape(-1)[0])
    ps = float(np.asarray(prob_scale).reshape(-1)[0])

    # quantize new K/V (exact same fp32 arithmetic as the reference)
    def quant(x, sc, off):
        return np.clip(
            np.round(x / np.float32(sc) + np.float32(off)), -128.0, 127.0
        ).astype(np.int8)

    k_q = quant(key, ks, ko)
    v_q = quant(value, ks, ko)
    flat = kv_cache.reshape(2, SLOTS, KVH, D).copy()
    flat[0, slot_mapping] = k_q
    flat[1, slot_mapping] = v_q
    cache = flat.reshape(2, NUM_BLOCKS, BLOCK, KVH, D)
    # gather pages -> per-request contiguous K/V, as exact ints minus offset
    k_eff = cache[0][block_table].reshape(B, S, KVH, D).astype(np.float32) - ko
    v_eff = cache[1][block_table].reshape(B, S, KVH, D).astype(np.float32) - ko
    k_eff = k_eff.astype(F16)
    v_eff = v_eff.astype(F16)

    q_int = np.clip(np.round(query / np.float32(qs) + np.float32(qo)), -128.0, 127.0)
    q_eff = (q_int.astype(np.float32) - qo).astype(F16)
    q5 = q_eff.reshape(B, LQ, KVH, G, D)

    # sanity: the device mask is affine-causal over the last LQ keys
    q_pos = seq_lens[:, None] - LQ + np.arange(LQ, dtype=np.int32)[None, :]
    k_pos = np.arange(S, dtype=np.int32)
    mask = (k_pos[None, None, :] <= q_pos[:, :, None]) & (
        k_pos[None, None, :] < seq_lens[:, None, None]
    )  # [B, LQ, S]
    assert mask[:, :, : S - LQ].all(), "prefix keys must be unmasked"
    jj = np.arange(LQ)[None, :]
    qq = np.arange(LQ)[:, None]
    want = (qq - jj) >= 0
    got = mask[:, :, S - LQ :]
    assert (got == want[None]).all(), "mask not affine-causal"

    C = float(qs * ks * SM_SCALE)
    inv_ps = float(1.0 / ps)
    out_scale = float(ps * ks)

    key_sig = (C, inv_ps, out_scale)
    if key_sig not in _CACHE:
        _CACHE[key_sig] = _build(C, inv_ps, out_scale)
    nc = _CACHE[key_sig]

    KQCOL = S + G * LQ
    in_maps = []
    for c in range(N_CORES):
        kqa = np.empty((B, 128, KQCOL), F16)
        # KT: [D, S] per b
        kqa[:, :, 0:S] = np.transpose(k_eff[:, :, c, :], (0, 2, 1))
        # QT: [D, G*LQ] per b
        kqa[:, :, S:] = q5[:, :, c, :, :].transpose(0, 3, 2, 1).reshape(B, 128, G * LQ)
        # V: [p-in-chunk, NCH*D] per b
        va = np.ascontiguousarray(
            v_eff[:, :, c, :]
            .reshape(B, NCH, 128, D)
            .transpose(0, 2, 1, 3)
            .reshape(B, 128, NCH * D)
        )
        in_maps.append({"kq": kqa, "vv": va})

    global _LAST_IN_MAPS
    _LAST_IN_MAPS = in_maps
    res = run_bass_kernel_spmd(nc, in_maps, list(range(N_CORES)))
    # res: [KVH][b, d, row*128+q] with row = g*2+lqc
    outs = np.stack([r["out"] for r in res.results])  # [KVH, B, D, 8*128]
    outs = outs.reshape(KVH, B, D, G, 2, 128)  # [kvh, b, d, g, lqc, q]
    full = (
        outs.transpose(1, 3, 4, 5, 0, 2)  # [b, g, lqc, q, kvh, d]
        .reshape(B, G, LQ, KVH, D)
        .transpose(0, 2, 3, 1, 4)  # [b, lq, kvh, g, d]
        .reshape(T, H, D)
    )
    return np.ascontiguousarray(full.astype(np.float32))


# revision 27
# speedup vs baseline: 1.0241x; 1.0241x over previous
"""Paged-KV-cache GQA attention with int8 tensor-cast quantization, TRN2.

Sharding: tensor-parallel over KV heads. Core c owns kv-head c and its G=4
query heads. Host side does only index-driven data movement (scatter new
quantized K/V into the paged cache, gather pages via block_table) plus the
int8 quantization of Q/K/V, all exact integer ops. The device kernel does
all the attention compute per (b, row) where row = (g, lqc) is a 128-query
block:

  scores[q, k] = sum_d Q[q,d] * K[k,d]           (PE, fp16 exact-int matmul
                                                  into 3 rotating PSUM regions)
  e = exp(C * scores)                            (ACT, PSUM->SBUF fp32,
                                                  accum_out -> row sums; the
                                                  causal tail gets raw exp,
                                                  masked + summed on DVE)
  p127 = RNE(e * 127/sum) -> int16 slab          (ONE DVE pass: the fp32->
                                                  int16 saturating convert IS
                                                  round-nearest-even)
  slab transpose p[q, k] -> pT[k, (c,row,q)]     (xbar DMA, int16 bytes, two
                                                  pieces per 4-row slab)
  in-place int16->fp16 convert of pT halves      (DVE, at the consuming row)
  outT[d, (row,q)] += V_c^T @ pT_c               (PE, V-stationary N=512,
                                                  interleaved with the next
                                                  batch's QK rows)

All matmul operands are integers held exactly in fp16 (|x| <= 128), so the
fp32 PSUM dot products are exact; the int16 convert reproduces
round-to-nearest-even of jnp.round for the prob quantization,
bit-identically to the reference chain (measured rel err 0.0).
"""

import sys

sys.path.insert(0, "/opt/trn_rl_repo")

import numpy as np

import concourse.bass as bass
import concourse.mybir as mybir
from concourse import tile
from concourse.bass_utils import run_bass_kernel_spmd

# Problem dims (hardcoded per spec)
B, H, KVH, D = 4, 32, 8, 128
LQ, S, BLOCK = 256, 4096, 16
BPS = S // BLOCK
NUM_BLOCKS = B * BPS
SLOTS = NUM_BLOCKS * BLOCK
T = B * LQ
G = H // KVH
N_CORES = 8
NCH = S // 128  # 32 partition-chunks of the key axis
NROW = G * (LQ // 128)  # 8 query rows of 128 per (b, core)
SM_SCALE = 1.0 / float(np.sqrt(D))
MAGIC = float(np.float32(12582912.0))  # 1.5 * 2**23: fp32 RNE round trick

F16 = np.float16
GS_SLAB = False  # gpsimd tensor_scalar measured 25x slower than DVE

_CACHE = {}


def _build(C, inv_ps, out_scale):
    nc = bass.Bass()
    f16 = mybir.dt.float16
    f32 = mybir.dt.float32
    i32 = mybir.dt.int32
    i16 = mybir.dt.int16
    AF = mybir.ActivationFunctionType
    OP = mybir.AluOpType
    X = mybir.AxisListType.X

    KQCOL = S + G * LQ

    kq = nc.dram_tensor("kq", [B, 128, KQCOL], f16, kind="ExternalInput")
    vv = nc.dram_tensor("vv", [B, 128, NCH * D], f16, kind="ExternalInput")
    # outT[b, d, row*128 + q] (row = g*2 + lqc), host transposes back
    out = nc.dram_tensor("out", [B, 128, NROW * 128], f32, kind="ExternalOutput")

    with tile.TileContext(nc) as tc:
        with (
            tc.tile_pool(name="const", bufs=1) as constp,
            tc.tile_pool(name="kqp", bufs=2) as kqp,
            tc.tile_pool(name="vp", bufs=2) as vp,
            tc.tile_pool(name="expf", bufs=2) as expp,
            tc.tile_pool(name="pslab", bufs=1) as pbfp,
            tc.tile_pool(name="ptslab", bufs=1) as ptp,
            tc.tile_pool(name="small", bufs=8) as smallp,
            tc.tile_pool(name="osb", bufs=1) as outp,
            tc.tile_pool(name="psum", bufs=1, space="PSUM") as psp,
        ):
            # additive causal masks for the last LQ key columns, one per
            # 128-row query half: keep iff q - j + lqc*128 >= 0
            ramp = constp.tile([128, 256], i32)
            nc.gpsimd.iota(ramp[:], pattern=[[-1, 256]], base=0, channel_multiplier=1)
            mask01 = []  # keep=1.0 / masked=0.0, for the last 256 key cols
            for lqc in range(LQ // 128):
                t = constp.tile([128, 256], f32, tag=f"mt{lqc}")
                nc.vector.tensor_scalar(
                    t[:], ramp[:], float(lqc * 128), 0.0, OP.add, OP.is_ge
                )
                mask01.append(t)

            # PSUM: sc = 3 rotating [128,1024] score regions (banks 0-5),
            # pv = 2 slab accumulators [128,512] (banks 6,7)
            sc = psp.tile([128, 3072], f32, tag="sc")
            pv = psp.tile([128, 1024], f32, tag="pv")

            kq_t = [None, None]
            v_t = [None, None]  # [cur b, prev b]
            pbf_t = [None, None]  # slab tiles of current b
            pt_t = [None, None]  # transposed slabs of previous b
            rctr = 0  # global score-region counter

            def emit_row(b, r, bt):
                """QK + softmax + quantize for query row r of batch b."""
                nonlocal rctr
                g, lqc = r // 2, r % 2
                qa = S + g * LQ + lqc * 128
                slots = [(rctr + j) % 3 for j in range(4)]
                rctr += 4

                def qk(j, half):
                    cs = j * 2 + half
                    col = slots[j] * 1024 + half * 512
                    nc.tensor.matmul(
                        sc[:, col : col + 512],
                        bt[:, qa : qa + 128],
                        bt[:, cs * 512 : (cs + 1) * 512],
                        start=True,
                        stop=True,
                        skip_group_check=True,
                    )

                # exp op plans: (src_col, width, e_col, accum?) — slot j3 ==
                # j0 always (3 regions, 4 uses), so exp#1 MUST be emitted
                # before the j3 matmuls reuse that region. Finer exp
                # granularity lets regions recycle piecewise (dissolves the
                # PE<->ACT region ping-pong). The last 256 key cols (causal
                # zone) get raw exp with NO accum; the mask + partial sum for
                # them run on DVE in SBUF, so no ACT op waits on a DVE op.
                if slots[0] == 2:  # pair (2,0) is address-non-contiguous
                    ops1 = [(2048, 1024, 0, 1), (0, 1024, 1024, 1)]
                else:
                    ops1 = [(slots[0] * 1024, 2048, 0, 1)]
                if slots[2] == 2:  # (slots[2], slots[3]) = (2, 0): split
                    ops2 = [(2048, 1024, 2048, 1), (0, 768, 3072, 1)]
                else:  # slots[3] == slots[2]+1: one contiguous 1792 read
                    ops2 = [(slots[2] * 1024, 1792, 2048, 1)]
                ops2 += [(slots[3] * 1024 + 768, 256, 3840, 0)]

                ef = expp.tile([128, S], f32, tag="ef")
                nsum = sum(o[3] for o in ops1 + ops2) + 1
                acc = smallp.tile([128, nsum], f32, tag="acc")
                nacc = 0

                def exp_ops(ops):
                    nonlocal nacc
                    for sc0, w, ec, has_acc in ops:
                        ao = None
                        if has_acc:
                            ao = acc[:, nacc : nacc + 1]
                            nacc += 1
                        nc.scalar.activation(
                            ef[:, ec : ec + w],
                            sc[:, sc0 : sc0 + w],
                            AF.Exp,
                            scale=C,
                            accum_out=ao,
                        )

                qk(0, 0); qk(0, 1); qk(1, 0); qk(1, 1)
                exp_ops(ops1)
                qk(2, 0); qk(2, 1); qk(3, 0); qk(3, 1)
                exp_ops(ops2)

                # causal mask: zero the masked tail of e, sum the kept part
                nc.vector.tensor_mul(
                    ef[:, 3840:4096], ef[:, 3840:4096], mask01[lqc][:]
                )
                nc.vector.tensor_reduce(
                    acc[:, nsum - 1 : nsum], ef[:, 3840:4096], X, OP.add
                )

                sumv = smallp.tile([128, 1], f32, tag="sumv")
                nc.vector.tensor_reduce(sumv[:], acc[:], X, OP.add)
                rv = smallp.tile([128, 1], f32, tag="rv")
                nc.vector.reciprocal(rv[:], sumv[:])
                r127 = smallp.tile([128, 1], f32, tag="r127")
                nc.vector.tensor_scalar_mul(r127[:], rv[:], inv_ps)

                # p127 = RNE(e * r127): the fp32->int16 saturating convert
                # rounds to nearest-even, straight to the slab; split into 2
                # chunk-halves so slab transposes can start piecewise
                s, gl = r // 4, r % 4
                for j in range(2):
                    nc.vector.tensor_scalar(
                        pbf_t[s][:, 16 * j : 16 * (j + 1), gl * 128 : (gl + 1) * 128],
                        ef[:, 2048 * j : 2048 * (j + 1)],
                        r127[:],
                        None,
                        OP.mult,
                    )

            def emit_pv(bprev, r, vtprev):
                """8 PV chunks of slab r//4 of the previous batch."""
                s = r // 4
                for c in range(8 * (r % 4), 8 * (r % 4) + 8):
                    nc.tensor.matmul(
                        pv[:, s * 512 : (s + 1) * 512],
                        vtprev[:, c * D : (c + 1) * D],
                        pt_t[s][:, c * 4 : (c + 1) * 4, :],
                        start=(c == 0),
                        stop=(c == NCH - 1),
                        skip_group_check=True,
                    )

            for b in range(B + 1):
                if b < B:
                    if b == 0:
                        bt = kqp.tile([128, KQCOL], f16, tag="kq", name="kqt")
                        nc.gpsimd.dma_start(out=bt[:], in_=kq[0])
                        kq_t = [bt, None]
                    bt = kq_t[0]
                    if b + 1 < B:
                        nxt = kqp.tile([128, KQCOL], f16, tag="kq", name="kqn")
                        nc.gpsimd.dma_start(out=nxt[:], in_=kq[b + 1])
                        kq_t = [nxt, None]
                    vt = vp.tile([128, NCH * D], f16, tag="v", name="vt")
                    nc.gpsimd.dma_start(out=vt[:], in_=vv[b])
                    v_t[1] = v_t[0]
                    v_t[0] = vt
                    pbf_t[0] = pbfp.tile([128, NCH, 512], i16, tag="pb0", name="pb0")
                    pbf_t[1] = pbfp.tile([128, NCH, 512], i16, tag="pb1", name="pb1")

                for r in range(NROW):
                    if b < B:
                        emit_row(b, r, bt)
                    if b > 0:
                        # int16->fp16 convert of the pt half the next two
                        # rows' PV consume (fine-grained: never head-of-line
                        # blocks the DVE queue on the slab transpose)
                        s, j = r // 4, r % 4
                        if j % 2 == 0:
                            h = j // 2
                            pc = pt_t[s][:, h * 64 : (h + 1) * 64, :]
                            nc.vector.tensor_copy(pc, pc.bitcast(i16))
                        emit_pv(b - 1, r, v_t[1] if b <= B - 1 else v_t[0])
                    # slab complete -> transpose it (int16 bytes) in 4
                    # chunk-quarter pieces for pipelining with cast/PV
                    if b < B and r % 4 == 3:
                        s = r // 4
                        pt_new = ptp.tile([128, 128, 128], f16, tag=f"pt{s}")
                        for j in range(2):
                            nc.sync.dma_start_transpose(
                                out=pt_new[:, 64 * j : 64 * (j + 1), :].bitcast(i16),
                                in_=pbf_t[s][:, 16 * j : 16 * (j + 1), :],
                            )
                        pt_t[s] = pt_new

                if b > 0:
                    ob = outp.tile([128, 1024], f32)
                    nc.vector.tensor_scalar_mul(ob[:], pv[:], out_scale)
                    nc.gpsimd.dma_start(out=out[b - 1], in_=ob[:])



    _legalize_waits(nc)
    return nc


def _legalize_waits(nc, maxw=1):
    """Walrus rejects instructions with too many sync waits. Move excess
    waits onto injected same-engine NoOps placed just before the
    instruction (engine program order preserved, so semantics identical)."""
    fixid = 0
    for bb in nc.main_func.blocks:
        insts = list(bb.instructions)
        changed = False
        newlist = []
        for ins in insts:
            si = ins.sync_info
            waits = list(si.on_wait) if si and si.on_wait else []
            if len(waits) > maxw:
                keep = waits[-maxw:]
                excess = waits[:-maxw]
                for j in range(0, len(excess), maxw):
                    nop = mybir.InstNoOp(name=f"I-waitfix-{fixid}", ins=[], outs=[])
                    fixid += 1
                    nop.engine = ins.engine
                    nop.sync_info = mybir.SyncInfo(
                        on_wait=excess[j : j + maxw], on_update=[]
                    )
                    newlist.append(nop)
                ins.sync_info = mybir.SyncInfo(
                    on_wait=keep,
                    on_update=list(si.on_update) if si.on_update else [],
                )
                changed = True
            newlist.append(ins)
        if changed:
            try:
                bb.instructions = newlist
            except Exception:
                bb.instructions.clear()
                bb.instructions.extend(newlist)
    return nc


def kernel(
    query,
    key,
    value,
    kv_cache,
    block_table,
    slot_mapping,
    query_start_loc,
    seq_lens,
    query_lens,
    q_scale,
    q_offset,
    kv_scale,
    kv_offset,
    prob_scale,
    prob_offset,
):
    query = np.asarray(query, np.float32)
    key = np.asarray(key, np.float32)
    value = np.asarray(value, np.float32)
    kv_cache = np.asarray(kv_cache)
    block_table = np.asarray(block_table, np.int32)
    slot_mapping = np.asarray(slot_mapping, np.int32)
    seq_lens = np.asarray(seq_lens, np.int32)
    qs = float(np.asarray(q_scale).reshape(-1)[0])
    qo = float(np.asarray(q_offset).reshape(-1)[0])
    ks = float(np.asarray(kv_scale).reshape(-1)[0])
    ko = float(np.asarray(kv_offset).reshape(-1)[0])
    ps = float(np.asarray(prob_scale).reshape(-1)[0])

    # quantize new K/V (exact same fp32 arithmetic as the reference)
    def quant(x, sc, off):
        return np.clip(
            np.round(x / np.float32(sc) + np.float32(off)), -128.0, 127.0
        ).astype(np.int8)

    k_q = quant(key, ks, ko)
    v_q = quant(value, ks, ko)
    flat = kv_cache.reshape(2, SLOTS, KVH, D).copy()
    flat[0, slot_mapping] = k_q
    flat[1, slot_mapping] = v_q
    cache = flat.reshape(2, NUM_BLOCKS, BLOCK, KVH, D)
    # gather pages -> per-request contiguous K/V, as exact ints minus offset
    k_eff = cache[0][block_table].reshape(B, S, KVH, D).astype(np.float32) - ko
    v_eff = cache[1][block_table].reshape(B, S, KVH, D).astype(np.float32) - ko
    k_eff = k_eff.astype(F16)
    v_eff = v_eff.astype(F16)

    q_int = np.clip(np.round(query / np.float32(qs) + np.float32(qo)), -128.0, 127.0)
    q_eff = (q_int.astype(np.float32) - qo).astype(F16)
    q5 = q_eff.reshape(B, LQ, KVH, G, D)

    # sanity: the device mask is affine-causal over the last LQ keys
    q_pos = seq_lens[:, None] - LQ + np.arange(LQ, dtype=np.int32)[None, :]
    k_pos = np.arange(S, dtype=np.int32)
    mask = (k_pos[None, None, :] <= q_pos[:, :, None]) & (
        k_pos[None, None, :] < seq_lens[:, None, None]
    )  # [B, LQ, S]
    assert mask[:, :, : S - LQ].all(), "prefix keys must be unmasked"
    jj = np.arange(LQ)[None, :]
    qq = np.arange(LQ)[:, None]
    want = (qq - jj) >= 0
    got = mask[:, :, S - LQ :]
    assert (got == want[None]).all(), "mask not affine-causal"

    C = float(qs * ks * SM_SCALE)
    inv_ps = float(1.0 / ps)
    out_scale = float(ps * ks)

    key_sig = (C, inv_ps, out_scale)
    if key_sig not in _CACHE:
        _CACHE[key_sig] = _build(C, inv_ps, out_scale)
    nc = _CACHE[key_sig]

    KQCOL = S + G * LQ
    in_maps = []
    for c in range(N_CORES):
        kqa = np.empty((B, 128, KQCOL), F16)
        # KT: [D, S] per b
        kqa[:, :, 0:S] = np.transpose(k_eff[:, :, c, :], (0, 2, 1))
        # QT: [D, G*LQ] per b
        kqa[:, :, S:] = q5[:, :, c, :, :].transpose(0, 3, 2, 1).reshape(B, 128, G * LQ)
        # V: [p-in-chunk, NCH*D] per b
        va = np.ascontiguousarray(
            v_eff[:, :, c, :]
            .reshape(B, NCH, 128, D)
            .transpose(0, 2, 1, 3)
            .reshape(B, 128, NCH * D)
        )
        in_maps.append({"kq": kqa, "vv": va})

    global _LAST_IN_MAPS
    _LAST_IN_MAPS = in_maps
    res = run_bass_kernel_spmd(nc, in_maps, list(range(N_CORES)))
    # res: [KVH][b, d, row*128+q] with row = g*2+lqc
    outs = np.stack([r["out"] for r in res.results])  # [KVH, B, D, 8*128]
    outs = outs.reshape(KVH, B, D, G, 2, 128)  # [kvh, b, d, g, lqc, q]
    full = (
        outs.transpose(1, 3, 4, 5, 0, 2)  # [b, g, lqc, q, kvh, d]
        .reshape(B, G, LQ, KVH, D)
        .transpose(0, 2, 3, 1, 4)  # [b, lq, kvh, g, d]
        .reshape(T, H, D)
    )
    return np.ascontiguousarray(full.astype(np.float32))
